# revision 32
# baseline (speedup 1.0000x reference)
"""Trainium2 Bass kernel for a Mamba block (embed lookup -> residual add ->
RMSNorm -> Mamba(in_proj, causal conv1d, selective scan, out_proj)).

Sharding: tensor-parallel over d_inner across 8 NeuronCores.
- preamble (embed gather + residual + RMSNorm) is token-sharded, then the
  normalized hidden states are AllGathered (bf16, d-major) so every core can
  run its d_inner shard of the Mamba block.
- x_proj partials are AllReduced per chunk (bf16) so the scan pipeline
  starts early.
- out_proj partials are ReduceScattered (bf16); host reassembles the output.

v2: per-chunk software pipeline; conv as diag-matmuls on PE + Silu on ACT;
single sentinel-chained scan instruction per (chunk, d-tile); all
intermediates (xc, x, zs) stay in SBUF.
"""

import numpy as np
import ml_dtypes

import concourse.bacc as bacc
import concourse.bass as bass
import concourse.mybir as mybir
import concourse.tile as tile
from concourse.bass import AP, IndirectOffsetOnAxis
from concourse.bass_utils import run_bass_kernel_spmd
from concourse.masks import make_identity

F32 = mybir.dt.float32
BF16 = mybir.dt.bfloat16
I32 = mybir.dt.int32
AF = mybir.ActivationFunctionType
ALU = mybir.AluOpType
EPS = 1e-5

BF = ml_dtypes.bfloat16


def _cfg(B, L, DM, DI, NST, DTR, DCONV, V, NC, LC, NRS, rs_f32=False):
    assert (B * L) % NC == 0 and DI % NC == 0
    c = dict(B=B, L=L, DM=DM, DI=DI, NST=NST, DTR=DTR, DCONV=DCONV, V=V,
             NC=NC, LC=LC, NRS=NRS, rs_f32=rs_f32)
    c["DSH"] = DI // NC           # channels per core
    c["TSH"] = (B * L) // NC      # tokens per core (preamble shard)
    c["TT"] = c["TSH"] // 128     # token tiles per core
    c["DT"] = c["DSH"] // 128     # channel tiles per core
    c["KT"] = DM // 128           # d_model k-tiles
    c["MT"] = 2 * c["DSH"] // 128  # xz column tiles
    c["OMT"] = DM // 128          # out_proj m tiles
    c["R2"] = DTR + 2 * NST
    c["NCH"] = B * (L // LC)      # number of scan chunks
    c["TPG"] = (B * L) // NRS     # tokens per reduce-scatter group
    assert (c["NCH"] * LC) % c["TPG"] == 0 and c["TPG"] % LC == 0
    assert c["TSH"] % 128 == 0 and c["DSH"] % 128 == 0 and L % LC == 0
    assert DTR <= 128 and 2 * NST <= 128
    return c


CFG = _cfg(B=2, L=2048, DM=2048, DI=4096, NST=16, DTR=128, DCONV=4, V=50257,
           NC=8, LC=256, NRS=4, rs_f32=False)


def build_nc(c, num_devices=None, reps=1):
    NC = c["NC"]
    B, L, DM, DI = c["B"], c["L"], c["DM"], c["DI"]
    NST, DTR, DCONV, V = c["NST"], c["DTR"], c["DCONV"], c["V"]
    DSH, TSH, TT, DT = c["DSH"], c["TSH"], c["TT"], c["DT"]
    KT, MT, OMT, R2 = c["KT"], c["MT"], c["OMT"], c["R2"]
    LC, NRS, TPG = c["LC"], c["NRS"], c["TPG"]
    BL = B * L
    MSH = DM // NC                # output rows per core after reduce-scatter
    RSDT = F32 if c["rs_f32"] else BF16
    NCH = c["NCH"]                # total scan chunks
    CPB = L // LC                 # chunks per batch entry
    groups = [list(range(NC))]
    LCH = LC + DCONV - 1          # conv tile with halo

    nc = bacc.Bacc("TRN2", target_bir_lowering=False, debug=False,
                   num_devices=num_devices or NC)

    # ---- kernel I/O ----
    ids_t = nc.dram_tensor("ids", [128, TT], I32, kind="ExternalInput")
    resid_t = nc.dram_tensor("resid", [TSH, DM], F32, kind="ExternalInput")
    embed_t = nc.dram_tensor("embed", [V, DM], F32, kind="ExternalInput")
    w_in_t = nc.dram_tensor("w_in", [DM, 2 * DSH], BF16, kind="ExternalInput")
    convw_t = nc.dram_tensor("convw", [128, DT * DCONV], F32, kind="ExternalInput")
    convb_t = nc.dram_tensor("convb", [128, DT], F32, kind="ExternalInput")
    xpw_t = nc.dram_tensor("xpw", [DSH, R2], BF16, kind="ExternalInput")
    dtw_t = nc.dram_tensor("dtw", [DTR, DSH], BF16, kind="ExternalInput")
    dtb_t = nc.dram_tensor("dtb", [128, DT], F32, kind="ExternalInput")
    A_t = nc.dram_tensor("A", [128, DT * NST], F32, kind="ExternalInput")
    Dp_t = nc.dram_tensor("Dp", [128, DT], F32, kind="ExternalInput")
    wo_t = nc.dram_tensor("wo", [DSH, DM], BF16, kind="ExternalInput")

    resid_out_t = nc.dram_tensor("resid_out", [TSH, DM], F32, kind="ExternalOutput")
    y_out_t = nc.dram_tensor("y_out", [NRS, MSH, TPG], F32, kind="ExternalOutput")

    with tile.TileContext(nc) as tc:
        with (
            tc.tile_pool(name="dram", bufs=1, space="DRAM") as dram,
            tc.tile_pool(name="const", bufs=1) as const,
        ):
            # ---- constants to SBUF ----
            ids_sb = const.tile([128, TT], I32)
            nc.sync.dma_start(ids_sb[:], ids_t[:])
            convw_sb = const.tile([128, DT * DCONV], F32)
            nc.sync.dma_start(convw_sb[:], convw_t[:])
            convb_sb = const.tile([128, DT], F32)
            nc.sync.dma_start(convb_sb[:], convb_t[:])
            dtb_sb = const.tile([128, DT], F32)
            nc.sync.dma_start(dtb_sb[:], dtb_t[:])
            A_sb = const.tile([128, DT * NST], F32)
            nc.sync.dma_start(A_sb[:], A_t[:])
            Dp_sb = const.tile([128, DT], F32)
            nc.sync.dma_start(Dp_sb[:], Dp_t[:])
            dtw_sb = const.tile([DTR, DSH], BF16)
            nc.sync.dma_start(dtw_sb[:], dtw_t[:])
            xpw_sb = const.tile([128, DT, R2], BF16)
            nc.sync.dma_start(xpw_sb[:], xpw_t[:].rearrange("(k p) r -> p k r", p=128))
            wo_sb = const.tile([128, DT, DM], BF16)
            nc.sync.dma_start(wo_sb[:], wo_t[:].rearrange("(k p) m -> p k m", p=128))
            w_sb = const.tile([128, KT, 2 * DSH], BF16)
            nc.sync.dma_start(w_sb[:], w_in_t[:].rearrange("(k p) m -> p k m", p=128))
            ident = const.tile([128, 128], BF16)
            make_identity(nc, ident[:])
            zero_b = const.tile([128, 1], F32)
            nc.vector.memset(zero_b[:], 0.0)
            eps_b = const.tile([128, 1], F32)
            nc.vector.memset(eps_b[:], EPS)
            one_b = const.tile([128, 1], F32)
            nc.vector.memset(one_b[:], 1.0)
            # conv taps as diagonal matrices (bf16) for PE depthwise conv
            cdiag = const.tile([128, DT * DCONV, 128], BF16)
            for dti in range(DT):
                for j in range(DCONV):
                    nc.vector.tensor_scalar(
                        cdiag[:, dti * DCONV + j, :], ident[:],
                        convw_sb[:, dti * DCONV + j:dti * DCONV + j + 1],
                        None, ALU.mult)

            for _rep in range(reps):
                # ---- internal DRAM ----
                hs_sh = dram.tile([DM, TSH], BF16, tag=f"hs_sh{_rep}", name=f"hs_sh{_rep}")
                hs_all = dram.tile([NC, DM, TSH], BF16, addr_space="Shared",
                                   tag=f"hs_all{_rep}", name=f"hs_all{_rep}")
                NPAIR = NCH // 2      # x_proj allreduce covers 2 chunks
                xdbl_par = [dram.tile([R2, 2 * LC], BF16, tag=f"xdp{g}_{_rep}",
                                      name=f"xdp{g}_{_rep}") for g in range(NPAIR)]
                xdbl = [dram.tile([R2, 2 * LC], BF16, addr_space="Shared",
                                  tag=f"xd{g}_{_rep}", name=f"xd{g}_{_rep}")
                        for g in range(NPAIR)]
                op_par = [dram.tile([DM, TPG], RSDT, tag=f"opp{g}_{_rep}",
                                    name=f"opp{g}_{_rep}") for g in range(NRS)]
                rs_out = [dram.tile([MSH, TPG], RSDT, tag=f"rso{g}_{_rep}",
                                    name=f"rso{g}_{_rep}") for g in range(NRS)]

                # ================= preamble: gather + residual + rmsnorm ========
                with (
                    tc.tile_pool(name="pre", bufs=3) as pre,
                    tc.tile_pool(name="pre_ps", bufs=2, space="PSUM") as pre_ps,
                ):
                    for j in range(TT):
                        emb = pre.tile([128, DM], F32, tag="emb")
                        nc.gpsimd.indirect_dma_start(
                            out=emb[:], out_offset=None, in_=embed_t[:],
                            in_offset=IndirectOffsetOnAxis(ap=ids_sb[:, j:j + 1], axis=0),
                        )
                        res = pre.tile([128, DM], F32, tag="res")
                        nc.sync.dma_start(res[:], resid_t[j * 128:(j + 1) * 128, :])
                        radd = pre.tile([128, DM], F32, tag="radd")
                        nc.vector.tensor_add(radd[:], emb[:], res[:])
                        nc.sync.dma_start(resid_out_t[j * 128:(j + 1) * 128, :], radd[:])
                        # rms scale = 1/sqrt(mean(x^2) + eps)
                        sq = pre.tile([128, DM], F32, tag="sq")
                        ss = pre.tile([128, 1], F32, tag="ss")
                        nc.scalar.activation(sq[:], radd[:], AF.Square, bias=zero_b[:, 0:1],
                                             accum_out=ss[:])
                        rr = pre.tile([128, 1], F32, tag="rr")
                        nc.scalar.activation(rr[:], ss[:], AF.Sqrt, bias=eps_b[:, 0:1],
                                             scale=1.0 / DM)
                        inv = pre.tile([128, 1], F32, tag="inv")
                        nc.vector.reciprocal(inv[:], rr[:])
                        hsb = pre.tile([128, DM], BF16, tag="hsb")
                        nc.vector.tensor_scalar_mul(hsb[:], radd[:], inv[:, 0:1])
                        # transpose to d-major and store the shard (one DMA)
                        stb = pre.tile([128, KT, 128], BF16, tag="stb")
                        for dcol in range(KT):
                            pt = pre_ps.tile([128, 128], BF16, tag="pt")
                            nc.tensor.transpose(pt[:], hsb[:, dcol * 128:(dcol + 1) * 128],
                                                ident[:])
                            nc.vector.tensor_copy(stb[:, dcol, :], pt[:])
                        nc.gpsimd.dma_start(
                            hs_sh[:, j * 128:(j + 1) * 128]
                            .rearrange("(k p) t -> p k t", p=128), stb[:])

                # ================= allgather hs ================================
                nc.gpsimd.collective_compute(
                    "AllGather", ALU.bypass, replica_groups=groups,
                    ins=[hs_sh[:].opt()], outs=[hs_all[:].opt()],
                )

                # ================= chunked pipeline ============================
                from contextlib import ExitStack
                with ExitStack() as stk:
                    hsA = stk.enter_context(tc.tile_pool(name="hsA", bufs=2))
                    psA = stk.enter_context(tc.tile_pool(name="psA", bufs=2, space="PSUM"))
                    xcp = stk.enter_context(tc.tile_pool(name="xcp", bufs=2))
                    zsp = stk.enter_context(tc.tile_pool(name="zsp", bufs=2))
                    psC = stk.enter_context(tc.tile_pool(name="psC", bufs=1, space="PSUM"))
                    xpp = stk.enter_context(tc.tile_pool(name="xp", bufs=2))
                    psX = stk.enter_context(tc.tile_pool(name="psX", bufs=1, space="PSUM"))
                    bcp = stk.enter_context(tc.tile_pool(name="bc", bufs=2))
                    scn = stk.enter_context(tc.tile_pool(name="scn", bufs=2))
                    psD = stk.enter_context(tc.tile_pool(name="psD", bufs=1, space="PSUM"))
                    dap = stk.enter_context(tc.tile_pool(name="dap", bufs=2))
                    ubp = stk.enter_context(tc.tile_pool(name="ubp", bufs=1))
                    htp = stk.enter_context(tc.tile_pool(name="htp", bufs=1))
                    cryp = stk.enter_context(tc.tile_pool(name="cryp", bufs=2))
                    yp = stk.enter_context(tc.tile_pool(name="yp", bufs=DT + 1))
                    psO = stk.enter_context(tc.tile_pool(name="psO", bufs=2, space="PSUM"))
                    oev = stk.enter_context(tc.tile_pool(name="oev", bufs=1))
                    cvt = stk.enter_context(tc.tile_pool(name="cvt", bufs=1))
                    carry_prev = [None] * DT
                    xc_prev = [None]
                    dtf_pair = [None]

                    def stage_a(b, ci):
                        """in_proj -> conv -> x_proj partial for one chunk."""
                        gc = b * CPB + ci
                        gt0 = b * L + ci * LC
                        tb, off = gt0 // TSH, gt0 % TSH
                        pr, po = gc // 2, (gc % 2) * LC
                        hst = hsA.tile([128, KT, LC], BF16, tag="hst", name="hst")
                        nc.sync.dma_start(
                            hst[:], hs_all[tb:tb + 1, :, off:off + LC]
                            .rearrange("o (k p) t -> p (o k) t", p=128))
                        xc = xcp.tile([128, DT, LCH], BF16, tag="xc", name="xc",
                                      bufs=3)
                        zs = zsp.tile([128, DT, LC], BF16, tag="zs", name="zs",
                                      bufs=4)
                        for m in range(MT):
                            ps = psA.tile([128, LC], F32, tag="ps", name="ps")
                            for k in range(KT):
                                nc.tensor.matmul(
                                    ps[:], lhsT=w_sb[:, k, m * 128:(m + 1) * 128],
                                    rhs=hst[:, k, :], start=(k == 0),
                                    stop=(k == KT - 1))
                            if m < DT:
                                nc.scalar.copy(xc[:, m, DCONV - 1:], ps[:])
                            else:
                                sg = xpp.tile([128, LC], BF16, tag="sg", name="sg",
                                              bufs=2)
                                nc.scalar.activation(sg[:], ps[:], AF.Sigmoid,
                                                     bias=zero_b[:, 0:1])
                                nc.vector.tensor_tensor(zs[:, m - DT, :], ps[:],
                                                        sg[:], ALU.mult)
                        # conv halo: previous chunk's last taps (zero at b start)
                        if ci == 0:
                            nc.vector.memset(xc[:, :, 0:DCONV - 1], 0.0)
                        else:
                            nc.vector.tensor_copy(xc[:, :, 0:DCONV - 1],
                                                  xc_prev[0][:, :, LC:LCH])
                        xc_prev[0] = xc

                        # causal depthwise conv1d + silu (PE + ACT)
                        xt4 = xpp.tile([128, DT, LC], BF16, tag="xt4", name="xt4",
                                       bufs=4)
                        for dti in range(DT):
                            acc = psC.tile([128, LC], F32, tag="acc", name="acc")
                            for j in range(DCONV):
                                nc.tensor.matmul(
                                    acc[:], lhsT=cdiag[:, dti * DCONV + j, :],
                                    rhs=xc[:, dti, j:j + LC], start=(j == 0),
                                    stop=(j == DCONV - 1))
                            sgc = xpp.tile([128, LC], BF16, tag="sg", name="sgc",
                                           bufs=2)
                            nc.scalar.activation(sgc[:], acc[:], AF.Sigmoid,
                                                 bias=convb_sb[:, dti:dti + 1])
                            nc.vector.scalar_tensor_tensor(
                                xt4[:, dti, :], acc[:], convb_sb[:, dti:dti + 1],
                                sgc[:], ALU.add, ALU.mult)

                        # x_proj partials (bf16)
                        ps1 = psX.tile([DTR, LC], F32, tag="ps1", name="ps1")
                        ps2 = psX.tile([2 * NST, LC], F32, tag="ps2", name="ps2")
                        for k in range(DT):
                            nc.tensor.matmul(ps1[:], lhsT=xpw_sb[:, k, 0:DTR],
                                             rhs=xt4[:, k, :], start=(k == 0),
                                             stop=(k == DT - 1))
                        for k in range(DT):
                            nc.tensor.matmul(ps2[:], lhsT=xpw_sb[:, k, DTR:R2],
                                             rhs=xt4[:, k, :], start=(k == 0),
                                             stop=(k == DT - 1))
                        s1 = xpp.tile([DTR, LC], BF16, tag="s1", name="s1")
                        nc.scalar.copy(s1[:], ps1[:])
                        s2 = xpp.tile([2 * NST, LC], BF16, tag="s2", name="s2")
                        nc.scalar.copy(s2[:], ps2[:])
                        nc.sync.dma_start(xdbl_par[pr][0:DTR, po:po + LC], s1[:])
                        nc.sync.dma_start(xdbl_par[pr][DTR:R2, po:po + LC], s2[:])
                        return xt4, zs

                    def stage_b(b, ci, xt4, zs):
                        """scan + out_proj for one chunk (after the pair AR)."""
                        gc = b * CPB + ci
                        g = (gc * LC) // TPG          # reduce-scatter group
                        gtok = gc * LC - g * TPG      # column offset in group
                        pr, po = gc // 2, (gc % 2) * LC
                        if True:
                            # ---- broadcast B and C rows across partitions
                            # (DMA replication straight from DRAM) ----
                            bc2 = bcp.tile([128, 2 * NST, LC], BF16, tag="bc2")
                            nc.sync.dma_start(
                                bc2[:], xdbl[pr][DTR:R2, po:po + LC]
                                .rearrange("(o n) l -> o n l", o=1)
                                .to_broadcast([128, 2 * NST, LC]))
                            bbc = bc2[:, 0:NST, :]
                            cbc = bc2[:, NST:, :]
                            # ---- dt softplus for the whole pair (batched
                            # Exp then Ln runs to minimize act-table loads) ----
                            if po == 0:
                                dtr16 = scn.tile([DTR, 2 * LC], BF16, tag="dtr16", bufs=1)
                                nc.sync.dma_start(dtr16[:], xdbl[pr][0:DTR, :])
                                dtes = []
                                for dti in range(DT):
                                    pdt = psD.tile([128, 2 * LC], F32, tag="pdt",
                                                   name="pdt")
                                    nc.tensor.matmul(
                                        pdt[:],
                                        lhsT=dtw_sb[:, dti * 128:(dti + 1) * 128],
                                        rhs=dtr16[:], start=True, stop=True)
                                    dtep = scn.tile([128, 2 * LC], BF16, tag="dtep",
                                                    name="dtep", bufs=4)
                                    nc.scalar.activation(dtep[:], pdt[:], AF.Exp,
                                                         bias=dtb_sb[:, dti:dti + 1])
                                    dtes.append(dtep)
                                dtfp = scn.tile([128, DT, 2 * LC], BF16, tag="dtfp",
                                                name="dtfp")
                                for dti in range(DT):
                                    nc.scalar.activation(dtfp[:, dti, :], dtes[dti][:],
                                                         AF.Ln, bias=one_b[:, 0:1])
                                dtf_pair[0] = dtfp
                            dtfp = dtf_pair[0]

                            yf_tiles = []
                            for dti in range(DT):
                                dtf = dtfp[:, dti, po:po + LC]
                                # u = dt * x
                                xt = xt4[:, dti, :]
                                ut = scn.tile([128, LC], BF16, tag="ut", bufs=1)
                                nc.vector.tensor_tensor(ut[:], dtf, xt, ALU.mult)
                                # dA' (col 0 is the scan-reset sentinel = 0)
                                dA = dap.tile([128, NST, LC + 1], BF16, tag="dA")
                                for n in range(NST):
                                    nc.scalar.activation(
                                        dA[:, n, 1:], dtf, AF.Exp,
                                        bias=zero_b[:, 0:1],
                                        scale=A_sb[:, dti * NST + n:dti * NST + n + 1])
                                nc.vector.memset(dA[:, :, 0:1], 0.0)
                                # uB' (col 0 carries the previous chunk's state)
                                uB = ubp.tile([128, NST, LC + 1], BF16, tag="uB")
                                u3 = ut[:].rearrange("p (o l) -> p o l", o=1) \
                                          .to_broadcast([128, NST, LC])
                                nc.vector.tensor_tensor(uB[:, :, 1:], u3, bbc, ALU.mult)
                                if ci == 0:
                                    nc.vector.memset(uB[:, :, 0:1], 0.0)
                                else:
                                    nc.vector.tensor_copy(uB[:, :, 0:1],
                                                          carry_prev[dti][:])
                                # single chained scan over all n sections
                                ht = htp.tile([128, NST, LC + 1], BF16, tag="ht")
                                nc.vector.tensor_tensor_scan(
                                    ht[:].rearrange("p n l -> p (n l)"),
                                    dA[:].rearrange("p n l -> p (n l)"),
                                    uB[:].rearrange("p n l -> p (n l)"),
                                    0.0, ALU.mult, ALU.add)
                                carry = cryp.tile([128, NST, 1], BF16, tag=f"carry{dti}")
                                nc.vector.tensor_copy(carry[:], ht[:, :, LC:LC + 1])
                                carry_prev[dti] = carry
                                # y = sum_n ht * C  (multiply then 16->1 tree add)
                                ym = ubp.tile([128, NST, LC], BF16, tag="ym")
                                nc.vector.tensor_tensor(ym[:], ht[:, :, 1:], cbc,
                                                        ALU.mult)
                                # tree-add scratch lives in the dead dA tile
                                t8 = dA[:, 0:8, 1:]
                                nc.vector.tensor_tensor(t8, ym[:, 0:8, :],
                                                        ym[:, 8:16, :], ALU.add)
                                t4 = dA[:, 8:12, 1:]
                                nc.vector.tensor_tensor(t4, t8[:, 0:4, :],
                                                        t8[:, 4:8, :], ALU.add)
                                t2 = dA[:, 12:14, 1:]
                                nc.vector.tensor_tensor(t2, t4[:, 0:2, :],
                                                        t4[:, 2:4, :], ALU.add)
                                yr = scn.tile([128, LC], BF16, tag="yr", bufs=1)
                                nc.vector.tensor_tensor(yr[:], dA[:, 12, 1:],
                                                        dA[:, 13, 1:], ALU.add)
                                # y += u * D ; y *= silu(z)
                                nc.vector.scalar_tensor_tensor(
                                    yr[:], xt, Dp_sb[:, dti:dti + 1], yr[:],
                                    ALU.mult, ALU.add)
                                yf = yp.tile([128, LC], BF16, tag="yf")
                                nc.vector.tensor_tensor(yf[:], yr[:], zs[:, dti, :],
                                                        ALU.mult)
                                yf_tiles.append(yf)

                            # ---- out_proj partial for this chunk (2 DMAs) ----
                            OH = max(1, OMT // 2)
                            for h in range(0, OMT, OH):
                                hn = min(OH, OMT - h)
                                ob = oev.tile([128, OH, LC], RSDT, tag="ob")
                                for mi in range(hn):
                                    m = h + mi
                                    pso = psO.tile([128, LC], F32, tag="pso")
                                    for k in range(DT):
                                        nc.tensor.matmul(
                                            pso[:],
                                            lhsT=wo_sb[:, k, m * 128:(m + 1) * 128],
                                            rhs=yf_tiles[k][:], start=(k == 0),
                                            stop=(k == DT - 1))
                                    nc.scalar.copy(ob[:, mi, :], pso[:])
                                nc.sync.dma_start(
                                    op_par[g][h * 128:(h + hn) * 128, gtok:gtok + LC]
                                    .rearrange("(m p) l -> p m l", p=128),
                                    ob[:, 0:hn, :])
                            # ---- reduce-scatter when a group completes ----
                            if (gc + 1) * LC % TPG == 0:
                                nc.gpsimd.collective_compute(
                                    "ReduceScatter", ALU.add, replica_groups=groups,
                                    ins=[op_par[g][:].opt()], outs=[rs_out[g][:].opt()],
                                )
                                if RSDT == F32:
                                    nc.sync.dma_start(
                                        y_out_t[g:g + 1],
                                        rs_out[g][:].rearrange("m t -> (m t)")
                                        .rearrange("(o m t) -> o m t", o=1, m=MSH))
                                else:
                                    for mm0 in range(0, MSH, 128):
                                        mm1 = min(mm0 + 128, MSH)
                                        mp = mm1 - mm0
                                        for t0 in range(0, TPG, 256):
                                            c16 = cvt.tile([mp, 256], BF16, tag="c16")
                                            nc.sync.dma_start(
                                                c16[:], rs_out[g][mm0:mm1, t0:t0 + 256])
                                            c32 = cvt.tile([mp, 256], F32, tag="c32")
                                            nc.vector.tensor_copy(c32[:], c16[:])
                                            nc.sync.dma_start(
                                                y_out_t[g, mm0:mm1, t0:t0 + 256], c32[:])

                    # ---- driver: pairs of chunks share one AllReduce; the
                    # next pair's stage_a + AR are issued BEFORE this pair's
                    # scan so the AR transfer hides under scan compute ----
                    pending = None
                    for b in range(B):
                        for cp in range(CPB // 2):
                            res_a = [stage_a(b, cp * 2 + half) for half in (0, 1)]
                            pr = (b * CPB + cp * 2) // 2
                            nc.gpsimd.collective_compute(
                                "AllReduce", ALU.add, replica_groups=groups,
                                ins=[xdbl_par[pr][:].opt()],
                                outs=[xdbl[pr][:].opt()],
                            )
                            if pending is not None:
                                pb_, pcp, pres = pending
                                for half in (0, 1):
                                    stage_b(pb_, pcp * 2 + half, *pres[half])
                            pending = (b, cp, res_a)
                    pb_, pcp, pres = pending
                    for half in (0, 1):
                        stage_b(pb_, pcp * 2 + half, *pres[half])
    nc.compile()
    return nc


# ===================== host-side sharding =====================

def make_in_maps(c, inputs):
    NC, DSH, TSH, DT = c["NC"], c["DSH"], c["TSH"], c["DT"]
    B, L, DM, DI = c["B"], c["L"], c["DM"], c["DI"]
    NST, DTR, DCONV, V = c["NST"], c["DTR"], c["DCONV"], c["V"]

    ids = np.asarray(inputs["input_ids"]).reshape(-1).astype(np.int32)
    resid = np.asarray(inputs["residual"], np.float32).reshape(B * L, DM)
    embed = np.ascontiguousarray(np.asarray(inputs["embed"], np.float32))
    norm_w = np.asarray(inputs["norm_w"], np.float32)
    w_in = np.asarray(inputs["in_proj_w"], np.float32) * norm_w[None, :]
    conv_w = np.asarray(inputs["conv_w"], np.float32)
    conv_b = np.asarray(inputs["conv_b"], np.float32)
    xpw = np.asarray(inputs["x_proj_w"], np.float32)
    dtw = np.asarray(inputs["dt_proj_w"], np.float32)
    dtb = np.asarray(inputs["dt_proj_b"], np.float32)
    A = (-np.exp(np.asarray(inputs["A_log"], np.float32))).astype(np.float32)
    Dp = np.asarray(inputs["D_param"], np.float32)
    wo = np.asarray(inputs["out_proj_w"], np.float32)

    in_maps = []
    for cc in range(NC):
        ch = slice(cc * DSH, (cc + 1) * DSH)
        w_x = w_in[cc * DSH:(cc + 1) * DSH, :]
        w_z = w_in[DI + cc * DSH:DI + (cc + 1) * DSH, :]
        w_c = np.concatenate([w_x, w_z], 0).T  # (DM, 2*DSH)
        cw = conv_w[ch].reshape(DT, 128, DCONV).transpose(1, 0, 2).reshape(128, DT * DCONV)
        cb = conv_b[ch].reshape(DT, 128).T
        dtb_c = dtb[ch].reshape(DT, 128).T
        A_c = A[ch].reshape(DT, 128, NST).transpose(1, 0, 2).reshape(128, DT * NST)
        Dp_c = Dp[ch].reshape(DT, 128).T
        in_maps.append({
            "ids": ids[cc * TSH:(cc + 1) * TSH].reshape(-1, 128).T.copy(),
            "resid": resid[cc * TSH:(cc + 1) * TSH].copy(),
            "embed": embed,
            "w_in": np.ascontiguousarray(w_c).astype(BF),
            "convw": np.ascontiguousarray(cw),
            "convb": np.ascontiguousarray(cb),
            "xpw": np.ascontiguousarray(xpw[:, ch].T).astype(BF),
            "dtw": np.ascontiguousarray(dtw[ch, :].T).astype(BF),
            "dtb": np.ascontiguousarray(dtb_c),
            "A": np.ascontiguousarray(A_c),
            "Dp": np.ascontiguousarray(Dp_c),
            "wo": np.ascontiguousarray(wo[:, ch].T).astype(BF),
        })
    return in_maps


def assemble(c, results):
    NC, TSH, DM, B, L = c["NC"], c["TSH"], c["DM"], c["B"], c["L"]
    NRS, TPG, MSH = c["NRS"], c["TPG"], c["DM"] // c["NC"]
    resid = np.concatenate([results[cc]["resid_out"] for cc in range(NC)], 0)
    y = np.stack([results[cc]["y_out"] for cc in range(NC)], 0)  # (NC,NRS,MSH,TPG)
    hs = y.transpose(1, 3, 0, 2).reshape(B * L, DM)
    return (hs.reshape(B, L, DM).astype(np.float32),
            resid.reshape(B, L, DM).astype(np.float32))


_COMPILED = {}


def get_compiled(c=None):
    key = id(c) if c is not None else "default"
    if key not in _COMPILED:
        _COMPILED[key] = build_nc(c or CFG)
    return _COMPILED[key]


def get_compiled_replicated(reps, c=None):
    key = ("rep", reps, id(c) if c is not None else "default")
    if key not in _COMPILED:
        _COMPILED[key] = build_nc(c or CFG, reps=reps)
    return _COMPILED[key], reps


def kernel(**inputs):
    c = CFG
    nc = get_compiled(c)
    in_maps = make_in_maps(c, inputs)
    res = run_bass_kernel_spmd(nc, in_maps, core_ids=list(range(c["NC"])))
    return assemble(c, res.results)


# revision 33
# speedup vs baseline: 1.1402x; 1.1402x over previous
"""Trainium2 Bass kernel for a Mamba block (embed lookup -> residual add ->
RMSNorm -> Mamba(in_proj, causal conv1d, selective scan, out_proj)).

Sharding: tensor-parallel over d_inner across 8 NeuronCores.
- preamble (embed gather + residual + RMSNorm) is token-sharded, then the
  normalized hidden states are AllGathered (bf16, d-major) so every core can
  run its d_inner shard of the Mamba block.
- x_proj partials are AllReduced per chunk (bf16) so the scan pipeline
  starts early.
- out_proj partials are ReduceScattered (bf16); host reassembles the output.

v2: per-chunk software pipeline; conv as diag-matmuls on PE + Silu on ACT;
single sentinel-chained scan instruction per (chunk, d-tile); all
intermediates (xc, x, zs) stay in SBUF.
"""

import numpy as np
import ml_dtypes

import concourse.bacc as bacc
import concourse.bass as bass
import concourse.mybir as mybir
import concourse.tile as tile
from concourse.bass import AP, IndirectOffsetOnAxis
from concourse.bass_utils import run_bass_kernel_spmd
from concourse.masks import make_identity

F32 = mybir.dt.float32
BF16 = mybir.dt.bfloat16
I32 = mybir.dt.int32
AF = mybir.ActivationFunctionType
ALU = mybir.AluOpType
EPS = 1e-5

BF = ml_dtypes.bfloat16


def _cfg(B, L, DM, DI, NST, DTR, DCONV, V, NC, LC, NRS, rs_f32=False):
    assert (B * L) % NC == 0 and DI % NC == 0
    c = dict(B=B, L=L, DM=DM, DI=DI, NST=NST, DTR=DTR, DCONV=DCONV, V=V,
             NC=NC, LC=LC, NRS=NRS, rs_f32=rs_f32)
    c["DSH"] = DI // NC           # channels per core
    c["TSH"] = (B * L) // NC      # tokens per core (preamble shard)
    c["TT"] = c["TSH"] // 128     # token tiles per core
    c["DT"] = c["DSH"] // 128     # channel tiles per core
    c["KT"] = DM // 128           # d_model k-tiles
    c["MT"] = 2 * c["DSH"] // 128  # xz column tiles
    c["OMT"] = DM // 128          # out_proj m tiles
    c["R2"] = DTR + 2 * NST
    c["NCH"] = B * (L // LC)      # number of scan chunks
    c["TPG"] = (B * L) // NRS     # tokens per reduce-scatter group
    assert (c["NCH"] * LC) % c["TPG"] == 0 and c["TPG"] % LC == 0
    assert c["TSH"] % 128 == 0 and c["DSH"] % 128 == 0 and L % LC == 0
    assert DTR <= 128 and 2 * NST <= 128
    return c


CFG = _cfg(B=2, L=2048, DM=2048, DI=4096, NST=16, DTR=128, DCONV=4, V=50257,
           NC=8, LC=256, NRS=4, rs_f32=False)


def build_nc(c, num_devices=None, reps=1):
    NC = c["NC"]
    B, L, DM, DI = c["B"], c["L"], c["DM"], c["DI"]
    NST, DTR, DCONV, V = c["NST"], c["DTR"], c["DCONV"], c["V"]
    DSH, TSH, TT, DT = c["DSH"], c["TSH"], c["TT"], c["DT"]
    KT, MT, OMT, R2 = c["KT"], c["MT"], c["OMT"], c["R2"]
    LC, NRS, TPG = c["LC"], c["NRS"], c["TPG"]
    BL = B * L
    MSH = DM // NC                # output rows per core after reduce-scatter
    RSDT = F32 if c["rs_f32"] else BF16
    NCH = c["NCH"]                # total scan chunks
    CPB = L // LC                 # chunks per batch entry
    groups = [list(range(NC))]
    LCH = LC + DCONV - 1          # conv tile with halo

    nc = bacc.Bacc("TRN2", target_bir_lowering=False, debug=False,
                   num_devices=num_devices or NC)

    # ---- kernel I/O ----
    ids_t = nc.dram_tensor("ids", [128, TT], I32, kind="ExternalInput")
    resid_t = nc.dram_tensor("resid", [TSH, DM], F32, kind="ExternalInput")
    embed_t = nc.dram_tensor("embed", [V, DM], F32, kind="ExternalInput")
    w_in_t = nc.dram_tensor("w_in", [DM, 2 * DSH], BF16, kind="ExternalInput")
    convw_t = nc.dram_tensor("convw", [128, DT * DCONV], F32, kind="ExternalInput")
    convb_t = nc.dram_tensor("convb", [128, DT], F32, kind="ExternalInput")
    xpw_t = nc.dram_tensor("xpw", [DSH, R2], BF16, kind="ExternalInput")
    dtw_t = nc.dram_tensor("dtw", [DTR, DSH], BF16, kind="ExternalInput")
    dtb_t = nc.dram_tensor("dtb", [128, DT], F32, kind="ExternalInput")
    A_t = nc.dram_tensor("A", [128, DT * NST], F32, kind="ExternalInput")
    Dp_t = nc.dram_tensor("Dp", [128, DT], F32, kind="ExternalInput")
    wo_t = nc.dram_tensor("wo", [DSH, DM], BF16, kind="ExternalInput")

    resid_out_t = nc.dram_tensor("resid_out", [TSH, DM], F32, kind="ExternalOutput")
    y_out_t = nc.dram_tensor("y_out", [NRS, MSH, TPG], F32, kind="ExternalOutput")

    with tile.TileContext(nc) as tc:
        with (
            tc.tile_pool(name="dram", bufs=1, space="DRAM") as dram,
            tc.tile_pool(name="const", bufs=1) as const,
        ):
            # ---- constants to SBUF ----
            ids_sb = const.tile([128, TT], I32)
            nc.sync.dma_start(ids_sb[:], ids_t[:])
            convw_sb = const.tile([128, DT * DCONV], F32)
            nc.sync.dma_start(convw_sb[:], convw_t[:])
            convb_sb = const.tile([128, DT], F32)
            nc.sync.dma_start(convb_sb[:], convb_t[:])
            dtb_sb = const.tile([128, DT], F32)
            nc.sync.dma_start(dtb_sb[:], dtb_t[:])
            A_sb = const.tile([128, DT * NST], F32)
            nc.sync.dma_start(A_sb[:], A_t[:])
            Dp_sb = const.tile([128, DT], F32)
            nc.sync.dma_start(Dp_sb[:], Dp_t[:])
            dtw_sb = const.tile([DTR, DSH], BF16)
            nc.sync.dma_start(dtw_sb[:], dtw_t[:])
            xpw_sb = const.tile([128, DT, R2], BF16)
            nc.sync.dma_start(xpw_sb[:], xpw_t[:].rearrange("(k p) r -> p k r", p=128))
            wo_sb = const.tile([128, DT, DM], BF16)
            nc.sync.dma_start(wo_sb[:], wo_t[:].rearrange("(k p) m -> p k m", p=128))
            w_sb = const.tile([128, KT, 2 * DSH], BF16)
            nc.sync.dma_start(w_sb[:], w_in_t[:].rearrange("(k p) m -> p k m", p=128))
            ident = const.tile([128, 128], BF16)
            make_identity(nc, ident[:])
            zero_b = const.tile([128, 1], F32)
            nc.vector.memset(zero_b[:], 0.0)
            eps_b = const.tile([128, 1], F32)
            nc.vector.memset(eps_b[:], EPS)
            one_b = const.tile([128, 1], F32)
            nc.vector.memset(one_b[:], 1.0)
            # conv taps as diagonal matrices (bf16) for PE depthwise conv
            cdiag = const.tile([128, DT * DCONV, 128], BF16)
            for dti in range(DT):
                for j in range(DCONV):
                    nc.vector.tensor_scalar(
                        cdiag[:, dti * DCONV + j, :], ident[:],
                        convw_sb[:, dti * DCONV + j:dti * DCONV + j + 1],
                        None, ALU.mult)

            for _rep in range(reps):
                # ---- internal DRAM ----
                hs_sh = dram.tile([DM, TSH], BF16, tag=f"hs_sh{_rep}", name=f"hs_sh{_rep}")
                hs_all = dram.tile([NC, DM, TSH], BF16, addr_space="Shared",
                                   tag=f"hs_all{_rep}", name=f"hs_all{_rep}")
                NPAIR = NCH // 2      # x_proj allreduce covers 2 chunks
                xdbl_par = [dram.tile([R2, 2 * LC], BF16, tag=f"xdp{g}_{_rep}",
                                      name=f"xdp{g}_{_rep}") for g in range(NPAIR)]
                xdbl = [dram.tile([R2, 2 * LC], BF16, addr_space="Shared",
                                  tag=f"xd{g}_{_rep}", name=f"xd{g}_{_rep}")
                        for g in range(NPAIR)]
                op_par = [dram.tile([DM, TPG], RSDT, tag=f"opp{g}_{_rep}",
                                    name=f"opp{g}_{_rep}") for g in range(NRS)]
                rs_out = [dram.tile([MSH, TPG], RSDT, tag=f"rso{g}_{_rep}",
                                    name=f"rso{g}_{_rep}") for g in range(NRS)]

                # ================= preamble: gather + residual + rmsnorm ========
                with (
                    tc.tile_pool(name="pre", bufs=3) as pre,
                    tc.tile_pool(name="pre_ps", bufs=2, space="PSUM") as pre_ps,
                ):
                    for j in range(TT):
                        emb = pre.tile([128, DM], F32, tag="emb")
                        nc.gpsimd.indirect_dma_start(
                            out=emb[:], out_offset=None, in_=embed_t[:],
                            in_offset=IndirectOffsetOnAxis(ap=ids_sb[:, j:j + 1], axis=0),
                        )
                        res = pre.tile([128, DM], F32, tag="res")
                        nc.sync.dma_start(res[:], resid_t[j * 128:(j + 1) * 128, :])
                        radd = pre.tile([128, DM], F32, tag="radd")
                        nc.vector.tensor_add(radd[:], emb[:], res[:])
                        nc.sync.dma_start(resid_out_t[j * 128:(j + 1) * 128, :], radd[:])
                        # rms scale = 1/sqrt(mean(x^2) + eps)
                        sq = pre.tile([128, DM], F32, tag="sq")
                        ss = pre.tile([128, 1], F32, tag="ss")
                        nc.scalar.activation(sq[:], radd[:], AF.Square, bias=zero_b[:, 0:1],
                                             accum_out=ss[:])
                        rr = pre.tile([128, 1], F32, tag="rr")
                        nc.scalar.activation(rr[:], ss[:], AF.Sqrt, bias=eps_b[:, 0:1],
                                             scale=1.0 / DM)
                        inv = pre.tile([128, 1], F32, tag="inv")
                        nc.vector.reciprocal(inv[:], rr[:])
                        hsb = pre.tile([128, DM], BF16, tag="hsb")
                        nc.vector.tensor_scalar_mul(hsb[:], radd[:], inv[:, 0:1])
                        # transpose to d-major and store the shard (one DMA)
                        stb = pre.tile([128, KT, 128], BF16, tag="stb")
                        for dcol in range(KT):
                            pt = pre_ps.tile([128, 128], BF16, tag="pt")
                            nc.tensor.transpose(pt[:], hsb[:, dcol * 128:(dcol + 1) * 128],
                                                ident[:])
                            nc.vector.tensor_copy(stb[:, dcol, :], pt[:])
                        nc.gpsimd.dma_start(
                            hs_sh[:, j * 128:(j + 1) * 128]
                            .rearrange("(k p) t -> p k t", p=128), stb[:])

                # ================= allgather hs ================================
                nc.gpsimd.collective_compute(
                    "AllGather", ALU.bypass, replica_groups=groups,
                    ins=[hs_sh[:].opt()], outs=[hs_all[:].opt()],
                )

                # ================= chunked pipeline ============================
                from contextlib import ExitStack
                with ExitStack() as stk:
                    hsA = stk.enter_context(tc.tile_pool(name="hsA", bufs=2))
                    psA = stk.enter_context(tc.tile_pool(name="psA", bufs=2, space="PSUM"))
                    xcp = stk.enter_context(tc.tile_pool(name="xcp", bufs=2))
                    zsp = stk.enter_context(tc.tile_pool(name="zsp", bufs=2))
                    psC = stk.enter_context(tc.tile_pool(name="psC", bufs=1, space="PSUM"))
                    xpp = stk.enter_context(tc.tile_pool(name="xp", bufs=2))
                    psX = stk.enter_context(tc.tile_pool(name="psX", bufs=1, space="PSUM"))
                    bcp = stk.enter_context(tc.tile_pool(name="bc", bufs=2))
                    scn = stk.enter_context(tc.tile_pool(name="scn", bufs=2))
                    psD = stk.enter_context(tc.tile_pool(name="psD", bufs=1, space="PSUM"))
                    dap = stk.enter_context(tc.tile_pool(name="dap", bufs=2))
                    ubp = stk.enter_context(tc.tile_pool(name="ubp", bufs=1))
                    htp = stk.enter_context(tc.tile_pool(name="htp", bufs=1))
                    cryp = stk.enter_context(tc.tile_pool(name="cryp", bufs=2))
                    yp = stk.enter_context(tc.tile_pool(name="yp", bufs=DT + 1))
                    psO = stk.enter_context(tc.tile_pool(name="psO", bufs=2, space="PSUM"))
                    oev = stk.enter_context(tc.tile_pool(name="oev", bufs=1))
                    cvt = stk.enter_context(tc.tile_pool(name="cvt", bufs=1))
                    carry_prev = [None] * DT
                    xc_prev = [None]
                    dtf_pair = [None]

                    def stage_a(b, ci):
                        """in_proj -> conv -> x_proj partial for one chunk."""
                        gc = b * CPB + ci
                        gt0 = b * L + ci * LC
                        tb, off = gt0 // TSH, gt0 % TSH
                        pr, po = gc // 2, (gc % 2) * LC
                        hst = hsA.tile([128, KT, LC], BF16, tag="hst", name="hst")
                        nc.sync.dma_start(
                            hst[:], hs_all[tb:tb + 1, :, off:off + LC]
                            .rearrange("o (k p) t -> p (o k) t", p=128))
                        xc = xcp.tile([128, DT, LCH], BF16, tag="xc", name="xc",
                                      bufs=3)
                        zs = zsp.tile([128, DT, LC], BF16, tag="zs", name="zs",
                                      bufs=4)
                        for m in range(MT):
                            ps = psA.tile([128, LC], F32, tag="ps", name="ps")
                            for k in range(KT):
                                nc.tensor.matmul(
                                    ps[:], lhsT=w_sb[:, k, m * 128:(m + 1) * 128],
                                    rhs=hst[:, k, :], start=(k == 0),
                                    stop=(k == KT - 1))
                            if m < DT:
                                nc.scalar.copy(xc[:, m, DCONV - 1:], ps[:])
                            else:
                                sg = xpp.tile([128, LC], BF16, tag="sg", name="sg",
                                              bufs=2)
                                nc.scalar.activation(sg[:], ps[:], AF.Sigmoid,
                                                     bias=zero_b[:, 0:1])
                                nc.vector.tensor_tensor(zs[:, m - DT, :], ps[:],
                                                        sg[:], ALU.mult)
                        # conv halo: previous chunk's last taps (zero at b start)
                        if ci == 0:
                            nc.vector.memset(xc[:, :, 0:DCONV - 1], 0.0)
                        else:
                            nc.vector.tensor_copy(xc[:, :, 0:DCONV - 1],
                                                  xc_prev[0][:, :, LC:LCH])
                        xc_prev[0] = xc

                        # causal depthwise conv1d + silu (PE + ACT)
                        xt4 = xpp.tile([128, DT, LC], BF16, tag="xt4", name="xt4",
                                       bufs=4)
                        for dti in range(DT):
                            acc = psC.tile([128, LC], F32, tag="acc", name="acc")
                            for j in range(DCONV):
                                nc.tensor.matmul(
                                    acc[:], lhsT=cdiag[:, dti * DCONV + j, :],
                                    rhs=xc[:, dti, j:j + LC], start=(j == 0),
                                    stop=(j == DCONV - 1))
                            sgc = xpp.tile([128, LC], BF16, tag="sg", name="sgc",
                                           bufs=2)
                            nc.scalar.activation(sgc[:], acc[:], AF.Sigmoid,
                                                 bias=convb_sb[:, dti:dti + 1])
                            nc.vector.scalar_tensor_tensor(
                                xt4[:, dti, :], acc[:], convb_sb[:, dti:dti + 1],
                                sgc[:], ALU.add, ALU.mult)

                        # x_proj partials (bf16)
                        ps1 = psX.tile([DTR, LC], F32, tag="ps1", name="ps1")
                        ps2 = psX.tile([2 * NST, LC], F32, tag="ps2", name="ps2")
                        for k in range(DT):
                            nc.tensor.matmul(ps1[:], lhsT=xpw_sb[:, k, 0:DTR],
                                             rhs=xt4[:, k, :], start=(k == 0),
                                             stop=(k == DT - 1))
                        for k in range(DT):
                            nc.tensor.matmul(ps2[:], lhsT=xpw_sb[:, k, DTR:R2],
                                             rhs=xt4[:, k, :], start=(k == 0),
                                             stop=(k == DT - 1))
                        s1 = xpp.tile([DTR, LC], BF16, tag="s1", name="s1")
                        nc.scalar.copy(s1[:], ps1[:])
                        s2 = xpp.tile([2 * NST, LC], BF16, tag="s2", name="s2")
                        nc.scalar.copy(s2[:], ps2[:])
                        nc.sync.dma_start(xdbl_par[pr][0:DTR, po:po + LC], s1[:])
                        nc.sync.dma_start(xdbl_par[pr][DTR:R2, po:po + LC], s2[:])
                        return xt4, zs

                    def stage_b(b, ci, xt4, zs):
                        """scan + out_proj for one chunk (after the pair AR)."""
                        gc = b * CPB + ci
                        g = (gc * LC) // TPG          # reduce-scatter group
                        gtok = gc * LC - g * TPG      # column offset in group
                        pr, po = gc // 2, (gc % 2) * LC
                        if True:
                            # ---- broadcast B and C rows across partitions ----
                            brow16 = scn.tile([1, 2 * NST, LC], BF16, tag="brow16",
                                              bufs=1)
                            nc.sync.dma_start(brow16[:], xdbl[pr][DTR:R2, po:po + LC]
                                              .rearrange("(o n) l -> o n l", o=1))
                            bc2 = bcp.tile([128, 2 * NST, LC], BF16, tag="bc2")
                            nc.gpsimd.partition_broadcast(bc2[:], brow16[:])
                            bbc = bc2[:, 0:NST, :]
                            cbc = bc2[:, NST:, :]
                            # ---- dt softplus for the whole pair (batched
                            # Exp then Ln runs to minimize act-table loads) ----
                            if po == 0:
                                dtr16 = scn.tile([DTR, 2 * LC], BF16, tag="dtr16", bufs=1)
                                nc.sync.dma_start(dtr16[:], xdbl[pr][0:DTR, :])
                                dtes = []
                                for dti in range(DT):
                                    pdt = psD.tile([128, 2 * LC], F32, tag="pdt",
                                                   name="pdt")
                                    nc.tensor.matmul(
                                        pdt[:],
                                        lhsT=dtw_sb[:, dti * 128:(dti + 1) * 128],
                                        rhs=dtr16[:], start=True, stop=True)
                                    dtep = scn.tile([128, 2 * LC], BF16, tag="dtep",
                                                    name="dtep", bufs=4)
                                    nc.scalar.activation(dtep[:], pdt[:], AF.Exp,
                                                         bias=dtb_sb[:, dti:dti + 1])
                                    dtes.append(dtep)
                                dtfp = scn.tile([128, DT, 2 * LC], BF16, tag="dtfp",
                                                name="dtfp")
                                for dti in range(DT):
                                    nc.scalar.activation(dtfp[:, dti, :], dtes[dti][:],
                                                         AF.Ln, bias=one_b[:, 0:1])
                                dtf_pair[0] = dtfp
                            dtfp = dtf_pair[0]

                            yf_tiles = []
                            for dti in range(DT):
                                dtf = dtfp[:, dti, po:po + LC]
                                # u = dt * x
                                xt = xt4[:, dti, :]
                                ut = scn.tile([128, LC], BF16, tag="ut", bufs=1)
                                nc.vector.tensor_tensor(ut[:], dtf, xt, ALU.mult)
                                # dA' (col 0 is the scan-reset sentinel = 0)
                                dA = dap.tile([128, NST, LC + 1], BF16, tag="dA")
                                for n in range(NST):
                                    nc.scalar.activation(
                                        dA[:, n, 1:], dtf, AF.Exp,
                                        bias=zero_b[:, 0:1],
                                        scale=A_sb[:, dti * NST + n:dti * NST + n + 1])
                                nc.vector.memset(dA[:, :, 0:1], 0.0)
                                # uB' (col 0 carries the previous chunk's state)
                                uB = ubp.tile([128, NST, LC + 1], BF16, tag="uB")
                                u3 = ut[:].rearrange("p (o l) -> p o l", o=1) \
                                          .to_broadcast([128, NST, LC])
                                nc.vector.tensor_tensor(uB[:, :, 1:], u3, bbc, ALU.mult)
                                if ci == 0:
                                    nc.vector.memset(uB[:, :, 0:1], 0.0)
                                else:
                                    nc.vector.tensor_copy(uB[:, :, 0:1],
                                                          carry_prev[dti][:])
                                # single chained scan over all n sections
                                ht = htp.tile([128, NST, LC + 1], BF16, tag="ht")
                                nc.vector.tensor_tensor_scan(
                                    ht[:].rearrange("p n l -> p (n l)"),
                                    dA[:].rearrange("p n l -> p (n l)"),
                                    uB[:].rearrange("p n l -> p (n l)"),
                                    0.0, ALU.mult, ALU.add)
                                carry = cryp.tile([128, NST, 1], BF16, tag=f"carry{dti}")
                                nc.vector.tensor_copy(carry[:], ht[:, :, LC:LC + 1])
                                carry_prev[dti] = carry
                                # y = sum_n ht * C  (multiply then 16->1 tree add)
                                ym = ubp.tile([128, NST, LC], BF16, tag="ym")
                                nc.vector.tensor_tensor(ym[:], ht[:, :, 1:], cbc,
                                                        ALU.mult)
                                # tree-add scratch lives in the dead dA tile
                                t8 = dA[:, 0:8, 1:]
                                nc.vector.tensor_tensor(t8, ym[:, 0:8, :],
                                                        ym[:, 8:16, :], ALU.add)
                                t4 = dA[:, 8:12, 1:]
                                nc.vector.tensor_tensor(t4, t8[:, 0:4, :],
                                                        t8[:, 4:8, :], ALU.add)
                                t2 = dA[:, 12:14, 1:]
                                nc.vector.tensor_tensor(t2, t4[:, 0:2, :],
                                                        t4[:, 2:4, :], ALU.add)
                                yr = scn.tile([128, LC], BF16, tag="yr", bufs=1)
                                nc.vector.tensor_tensor(yr[:], dA[:, 12, 1:],
                                                        dA[:, 13, 1:], ALU.add)
                                # y += u * D ; y *= silu(z)
                                nc.vector.scalar_tensor_tensor(
                                    yr[:], xt, Dp_sb[:, dti:dti + 1], yr[:],
                                    ALU.mult, ALU.add)
                                yf = yp.tile([128, LC], BF16, tag="yf")
                                nc.vector.tensor_tensor(yf[:], yr[:], zs[:, dti, :],
                                                        ALU.mult)
                                yf_tiles.append(yf)

                            # ---- out_proj partial for this chunk (2 DMAs) ----
                            OH = max(1, OMT // 2)
                            for h in range(0, OMT, OH):
                                hn = min(OH, OMT - h)
                                ob = oev.tile([128, OH, LC], RSDT, tag="ob")
                                for mi in range(hn):
                                    m = h + mi
                                    pso = psO.tile([128, LC], F32, tag="pso")
                                    for k in range(DT):
                                        nc.tensor.matmul(
                                            pso[:],
                                            lhsT=wo_sb[:, k, m * 128:(m + 1) * 128],
                                            rhs=yf_tiles[k][:], start=(k == 0),
                                            stop=(k == DT - 1))
                                    nc.scalar.copy(ob[:, mi, :], pso[:])
                                nc.sync.dma_start(
                                    op_par[g][h * 128:(h + hn) * 128, gtok:gtok + LC]
                                    .rearrange("(m p) l -> p m l", p=128),
                                    ob[:, 0:hn, :])
                            # ---- reduce-scatter when a group completes ----
                            if (gc + 1) * LC % TPG == 0:
                                nc.gpsimd.collective_compute(
                                    "ReduceScatter", ALU.add, replica_groups=groups,
                                    ins=[op_par[g][:].opt()], outs=[rs_out[g][:].opt()],
                                )
                                if RSDT == F32:
                                    nc.sync.dma_start(
                                        y_out_t[g:g + 1],
                                        rs_out[g][:].rearrange("m t -> (m t)")
                                        .rearrange("(o m t) -> o m t", o=1, m=MSH))
                                else:
                                    for mm0 in range(0, MSH, 128):
                                        mm1 = min(mm0 + 128, MSH)
                                        mp = mm1 - mm0
                                        for t0 in range(0, TPG, 256):
                                            c16 = cvt.tile([mp, 256], BF16, tag="c16")
                                            nc.sync.dma_start(
                                                c16[:], rs_out[g][mm0:mm1, t0:t0 + 256])
                                            c32 = cvt.tile([mp, 256], F32, tag="c32")
                                            nc.vector.tensor_copy(c32[:], c16[:])
                                            nc.sync.dma_start(
                                                y_out_t[g, mm0:mm1, t0:t0 + 256], c32[:])

                    # ---- driver: pairs of chunks share one AllReduce; the
                    # next pair's stage_a + AR are issued BEFORE this pair's
                    # scan so the AR transfer hides under scan compute ----
                    pending = None
                    for b in range(B):
                        for cp in range(CPB // 2):
                            res_a = [stage_a(b, cp * 2 + half) for half in (0, 1)]
                            pr = (b * CPB + cp * 2) // 2
                            nc.gpsimd.collective_compute(
                                "AllReduce", ALU.add, replica_groups=groups,
                                ins=[xdbl_par[pr][:].opt()],
                                outs=[xdbl[pr][:].opt()],
                            )
                            if pending is not None:
                                pb_, pcp, pres = pending
                                for half in (0, 1):
                                    stage_b(pb_, pcp * 2 + half, *pres[half])
                            pending = (b, cp, res_a)
                    pb_, pcp, pres = pending
                    for half in (0, 1):
                        stage_b(pb_, pcp * 2 + half, *pres[half])
    nc.compile()
    return nc


# ===================== host-side sharding =====================

def make_in_maps(c, inputs):
    NC, DSH, TSH, DT = c["NC"], c["DSH"], c["TSH"], c["DT"]
    B, L, DM, DI = c["B"], c["L"], c["DM"], c["DI"]
    NST, DTR, DCONV, V = c["NST"], c["DTR"], c["DCONV"], c["V"]

    ids = np.asarray(inputs["input_ids"]).reshape(-1).astype(np.int32)
    resid = np.asarray(inputs["residual"], np.float32).reshape(B * L, DM)
    embed = np.ascontiguousarray(np.asarray(inputs["embed"], np.float32))
    norm_w = np.asarray(inputs["norm_w"], np.float32)
    w_in = np.asarray(inputs["in_proj_w"], np.float32) * norm_w[None, :]
    conv_w = np.asarray(inputs["conv_w"], np.float32)
    conv_b = np.asarray(inputs["conv_b"], np.float32)
    xpw = np.asarray(inputs["x_proj_w"], np.float32)
    dtw = np.asarray(inputs["dt_proj_w"], np.float32)
    dtb = np.asarray(inputs["dt_proj_b"], np.float32)
    A = (-np.exp(np.asarray(inputs["A_log"], np.float32))).astype(np.float32)
    Dp = np.asarray(inputs["D_param"], np.float32)
    wo = np.asarray(inputs["out_proj_w"], np.float32)

    in_maps = []
    for cc in range(NC):
        ch = slice(cc * DSH, (cc + 1) * DSH)
        w_x = w_in[cc * DSH:(cc + 1) * DSH, :]
        w_z = w_in[DI + cc * DSH:DI + (cc + 1) * DSH, :]
        w_c = np.concatenate([w_x, w_z], 0).T  # (DM, 2*DSH)
        cw = conv_w[ch].reshape(DT, 128, DCONV).transpose(1, 0, 2).reshape(128, DT * DCONV)
        cb = conv_b[ch].reshape(DT, 128).T
        dtb_c = dtb[ch].reshape(DT, 128).T
        A_c = A[ch].reshape(DT, 128, NST).transpose(1, 0, 2).reshape(128, DT * NST)
        Dp_c = Dp[ch].reshape(DT, 128).T
        in_maps.append({
            "ids": ids[cc * TSH:(cc + 1) * TSH].reshape(-1, 128).T.copy(),
            "resid": resid[cc * TSH:(cc + 1) * TSH].copy(),
            "embed": embed,
            "w_in": np.ascontiguousarray(w_c).astype(BF),
            "convw": np.ascontiguousarray(cw),
            "convb": np.ascontiguousarray(cb),
            "xpw": np.ascontiguousarray(xpw[:, ch].T).astype(BF),
            "dtw": np.ascontiguousarray(dtw[ch, :].T).astype(BF),
            "dtb": np.ascontiguousarray(dtb_c),
            "A": np.ascontiguousarray(A_c),
            "Dp": np.ascontiguousarray(Dp_c),
            "wo": np.ascontiguousarray(wo[:, ch].T).astype(BF),
        })
    return in_maps


def assemble(c, results):
    NC, TSH, DM, B, L = c["NC"], c["TSH"], c["DM"], c["B"], c["L"]
    NRS, TPG, MSH = c["NRS"], c["TPG"], c["DM"] // c["NC"]
    resid = np.concatenate([results[cc]["resid_out"] for cc in range(NC)], 0)
    y = np.stack([results[cc]["y_out"] for cc in range(NC)], 0)  # (NC,NRS,MSH,TPG)
    hs = y.transpose(1, 3, 0, 2).reshape(B * L, DM)
    return (hs.reshape(B, L, DM).astype(np.float32),
            resid.reshape(B, L, DM).astype(np.float32))


_COMPILED = {}


def get_compiled(c=None):
    key = id(c) if c is not None else "default"
    if key not in _COMPILED:
        _COMPILED[key] = build_nc(c or CFG)
    return _COMPILED[key]


def get_compiled_replicated(reps, c=None):
    key = ("rep", reps, id(c) if c is not None else "default")
    if key not in _COMPILED:
        _COMPILED[key] = build_nc(c or CFG, reps=reps)
    return _COMPILED[key], reps


def kernel(**inputs):
    c = CFG
    nc = get_compiled(c)
    in_maps = make_in_maps(c, inputs)
    res = run_bass_kernel_spmd(nc, in_maps, core_ids=list(range(c["NC"])))
    return assemble(c, res.results)
